# revision 4
# baseline (speedup 1.0000x reference)
"""7x7 grayscale dilation (flat SE, zero padding) on Trainium2, 8 NeuronCores.

Strategy (pure data parallel, batch-sharded, 12 images of 512x512 per core):
  - bf16 end-to-end on device (host converts f32<->bf16; max is exact in bf16,
    only the input rounding contributes error ~0.4% << 2% tolerance).
    bf16 halves DMA bytes and doubles DVE throughput (2x_1p perf mode).
  - images are padded host-side to 528 rows (3 zero halo rows + 13 zero pad,
    16-aligned for the XBAR) and loaded TRANSPOSED via the DMA crossbar
    (dma_start_transpose), so the vertical 7-max cascade runs along the free
    dim right away with its halo baked in.
  - one PE transpose (bf16 identity matmul -> bf16 PSUM) per image, one big
    ACT copy to evacuate into the column-haloed z tile, then the horizontal
    cascade and a plain store.
  - shift-max cascade (shifts 1,2,3) = 3 DVE tensor_tensor ops per direction.
  - software pipelining: V(g+1) is emitted before H(g) so the DVE stays busy
    while PE/ACT of group g run; head/tail ops are split finer.
"""
import numpy as np

_CACHE = {}

N_CORES = 8
IMGS = 12  # images per core: 4 batches x 3 channels
H = W = 512
HPAD = 528  # rows padded host-side: 3 zero halo + 512 data + 13 zero (16-aligned)


def _build_nc(group=2, nslot=4, p_bufs=2, headsplit=True, tailsplit=True, pe_halves=1, nslot_auz=None, v0_fine=8, v0_w=2):
    from contextlib import ExitStack
    from concourse import bacc, tile, mybir
    from concourse.masks import make_identity

    F32 = mybir.dt.float32
    BF16 = mybir.dt.bfloat16
    MAX = mybir.AluOpType.max
    G = group
    NG = IMGS // G
    FD = 4 * G
    if nslot_auz is None:
        nslot_auz = nslot
    assert group % pe_halves == 0, (group, pe_halves)

    nc = bacc.Bacc("TRN2", target_bir_lowering=False)
    x_in = nc.dram_tensor("x", [IMGS, HPAD, W], BF16, kind="ExternalInput")
    y_out = nc.dram_tensor("y", [IMGS, H, W], BF16, kind="ExternalOutput")

    with tile.TileContext(nc) as tc:
        with ExitStack() as ctx:
            pool = ctx.enter_context(tc.tile_pool(name="p", bufs=1))
            psum = ctx.enter_context(tc.tile_pool(name="ps", bufs=p_bufs, space="PSUM"))

            ident = pool.tile([128, 128], BF16)
            make_identity(nc, ident[:])

            xts, auzs = [], []
            for s in range(nslot):
                b_xt = pool.tile([128, FD, HPAD], BF16, tag=f"xt{s}")
                xts.append(b_xt)
            for s in range(nslot_auz):
                b_a = pool.tile([128, FD, 517], BF16, tag=f"a{s}")
                b_u = pool.tile([128, FD, 517], BF16, tag=f"u{s}")
                b_z = pool.tile([128, FD, 518], BF16, tag=f"z{s}")
                # persistent zero column halos for z; never rewritten
                nc.gpsimd.memset(b_z[:, :, 0:3], 0.0)
                nc.gpsimd.memset(b_z[:, :, 515:518], 0.0)
                auzs.append((b_a, b_u, b_z))

            def slot(g):
                return (xts[g % nslot],) + auzs[g % nslot_auz]

            def emit_loads(g, per_strip=False):
                b_xt = xts[g % nslot]
                for li in range(G):
                    i = g * G + li
                    # whole-image transposed load (contiguous dest at offset
                    # 0): b_xt[p, 4li+t, rp] = x[i][rp, t*128+p], rp = padded
                    # row; halo zeros are baked into the host pad.
                    if per_strip:
                        for t in range(4):
                            nc.sync.dma_start_transpose(
                                b_xt[:, 4 * li + t, :],
                                x_in[i][:, 128 * t : 128 * t + 128],
                            )
                    else:
                        nc.sync.dma_start_transpose(
                            b_xt[:, 4 * li : 4 * li + 4, :], x_in[i]
                        )

            def emit_V(g, split=1, pieces=None):
                b_xt, b_a, b_u, b_z = slot(g)
                if pieces is None:
                    step = max(1, FD // split)
                    pieces = []
                    for c0 in range(0, FD, step):
                        pieces.append((c0, min(c0 + step, FD)))
                for c0, c1 in pieces:
                    nc.vector.tensor_tensor(
                        b_a[:, c0:c1, 0:517], b_xt[:, c0:c1, 0:517],
                        b_xt[:, c0:c1, 1:518], op=MAX)
                    nc.vector.tensor_tensor(
                        b_u[:, c0:c1, 0:515], b_a[:, c0:c1, 0:515],
                        b_a[:, c0:c1, 2:517], op=MAX)
                    nc.vector.tensor_tensor(
                        b_a[:, c0:c1, 0:512], b_u[:, c0:c1, 0:512],
                        b_u[:, c0:c1, 3:515], op=MAX)

            def emit_PE(g):
                b_xt, b_a, b_u, b_z = slot(g)
                gh = G // pe_halves
                for h in range(pe_halves):
                    Pt = psum.tile([128, 4 * gh, 512], BF16, tag="P")
                    for lj in range(gh):
                        li = h * gh + lj
                        for b in range(4):      # output row blocks
                            for t1 in range(4):  # column strips
                                nc.tensor.matmul(
                                    Pt[:, 4 * lj + b, 128 * t1 : 128 * t1 + 128],
                                    b_a[:, 4 * li + t1, 128 * b : 128 * b + 128],
                                    ident[:],
                                    is_transpose=True,
                                )
                    nc.scalar.copy(
                        b_z[:, 4 * gh * h : 4 * gh * (h + 1), 3:515], Pt[:]
                    )

            def emit_stores(g, c0=0, c1=None):
                if c1 is None:
                    c1 = FD
                b_u = slot(g)[2]
                for li in range(G):
                    lo, hi = max(c0, 4 * li), min(c1, 4 * li + 4)
                    if lo >= hi:
                        continue
                    i = g * G + li
                    eng = nc.scalar if li % 2 == 0 else nc.sync
                    eng.dma_start(
                        y_out[i].rearrange("(t p) w -> p t w", p=128, t=4)[
                            :, lo - 4 * li : hi - 4 * li, :
                        ],
                        b_u[:, lo:hi, 0:512],
                    )

            def emit_H(g, split=1, with_store=False, pieces=None):
                b_xt, b_a, b_u, b_z = slot(g)
                if pieces is None:
                    step = max(1, FD // split)
                    pieces = []
                    for c0 in range(0, FD, step):
                        pieces.append((c0, min(c0 + step, FD)))
                for c0, c1 in pieces:
                    nc.vector.tensor_tensor(
                        b_u[:, c0:c1, 0:517], b_z[:, c0:c1, 0:517],
                        b_z[:, c0:c1, 1:518], op=MAX)
                    nc.vector.tensor_tensor(
                        b_a[:, c0:c1, 0:515], b_u[:, c0:c1, 0:515],
                        b_u[:, c0:c1, 2:517], op=MAX)
                    nc.vector.tensor_tensor(
                        b_u[:, c0:c1, 0:512], b_a[:, c0:c1, 0:512],
                        b_a[:, c0:c1, 3:515], op=MAX)
                    if with_store:
                        emit_stores(g, c0, c1)

            # software pipeline: V(g+1) before H(g)
            emit_loads(0, per_strip=headsplit)
            if NG > 1:
                emit_loads(1)
            if headsplit:
                # fine pieces while loads stream in (DVE is load-gated),
                # wider after
                pieces, c0 = [], 0
                while c0 < FD:
                    w = 1 if c0 < v0_fine else v0_w
                    c1 = min(c0 + w, FD)
                    pieces.append((c0, c1))
                    c0 = c1
                emit_V(0, pieces=pieces)
            else:
                emit_V(0, split=1)
            for g in range(NG):
                last = g == NG - 1
                if g + 2 < NG:
                    emit_loads(g + 2)
                emit_PE(g)
                if g + 1 < NG:
                    emit_V(g + 1, split=(2 if (headsplit and g == 0) else 1))
                if tailsplit and last:
                    # shrink pieces toward the end so the final store is small
                    pieces = [(c0, min(c0 + 2, FD)) for c0 in range(0, FD - 2, 2)]
                    pieces.append((FD - 2, FD - 1))
                    pieces.append((FD - 1, FD))
                    emit_H(g, with_store=True, pieces=pieces)
                else:
                    emit_H(g, split=1)
                    emit_stores(g)

    nc.finalize()
    return nc


def _get_nc():
    if "nc" not in _CACHE:
        _CACHE["nc"] = _build_nc(group=3, nslot=3, nslot_auz=3, pe_halves=3, v0_fine=5, v0_w=3)
    return _CACHE["nc"]


def _run_bass(x, trace=False):
    """x: (32,3,512,512) float32 -> (32,3,512,512) float32 via 8 cores."""
    import ml_dtypes
    from concourse.bass_utils import run_bass_kernel_spmd

    nc = _get_nc()
    xr = np.ascontiguousarray(x).reshape(N_CORES, IMGS, H, W).astype(ml_dtypes.bfloat16)
    xp = np.zeros((N_CORES, IMGS, HPAD, W), dtype=ml_dtypes.bfloat16)
    xp[:, :, 3 : 3 + H, :] = xr
    in_maps = [{"x": xp[k]} for k in range(N_CORES)]
    r = run_bass_kernel_spmd(nc, in_maps, list(range(N_CORES)), trace=trace)
    out = np.stack(
        [np.asarray(r.results[k]["y"]).astype(np.float32) for k in range(N_CORES)],
        axis=0,
    )
    return out.reshape(32, 3, 512, 512), r


def kernel(x, se):
    x = np.asarray(x, dtype=np.float32)
    se = np.asarray(se, dtype=np.float32)
    if se.shape == (7, 7) and np.all(se == 1.0):
        out, _ = _run_bass(x)
        return out
    # general fallback (never hit for this problem's inputs)
    kh, kw = se.shape
    ph, pw = kh // 2, kw // 2
    bias = se.reshape(-1) - 1.0
    mask = (bias >= 0).astype(x.dtype)
    xp = np.pad(x, ((0, 0), (0, 0), (ph, ph), (pw, pw)))
    out = np.full(x.shape, -np.inf, dtype=x.dtype)
    for i in range(kh * kw):
        r, c = i // kw, i % kw
        win = xp[:, :, r : r + x.shape[2], c : c + x.shape[3]]
        out = np.maximum(out, mask[i] * win + bias[i])
    return out
